# revision 34
# baseline (speedup 1.0000x reference)
"""ARMA GNN (3x ARMAConv K=2,T=2 + global mean pool + linear) on 8 trn2
NeuronCores.

Strategy (dst-sharded message passing with a replicated bf16 feature table):
  - Nodes sharded by dst across 8 cores (6250 each). Each inner ARMA
    iteration rebuilds a [65536, 128] bf16 node-feature table (rows
    pre-scaled by dinv[src]) via AllGather of per-core [8192, 128] chunks.
  - Per-core aggregation: dma_gather of the in-edge source rows (slot
    schedule built on host: per dst node, its edges padded to a pow2 run
    degree D; nodes grouped into equal-D runs so a static [128, 128/D]
    block-ones bf16 pattern reduces each 128-slot tile with one
    TensorEngine matmul into per-node PSUM columns).
  - gcn norm factorized: dinv[src] pre-scale (table), dinv[dst] post-scale.
  - dma_gather indices are int16, so sources are split into two 32768-row
    windows (cores 0-3 / 4-7); each window has its own run schedule. The
    window-B partial aggregate (in B-run column order) is transposed to an
    HBM scratch and gathered back in A-column order, then added.
  - Dense matmuls keep features on partitions (T-layout), weights as lhsT.
  - Mean pool via matmul with host-built (0.5/count)-weighted pool matrix,
    AllReduce, small linear head.
"""
import math
import os

import numpy as np
import ml_dtypes

import concourse.bacc as bacc
import concourse.mybir as mybir
import concourse.tile as tile
from concourse.bass_utils import run_bass_kernel_spmd

N = 50000
E = 800000
G = 64
F_IN = 64
H = 64
KS = 2
TS = 2
OUT = 24
NC = 8
SH = N // NC
P = 128
FEAT = KS * H          # 128
CHUNK_ROWS = 6912
DS = [1, 2, 4, 8, 16, 32, 64, 128]
GCH = 22               # gather chunk, in 128-slot tiles
FIXCH = 8              # fixup gather chunk, in 128-col blocks

bf16 = mybir.dt.bfloat16
f32 = mybir.dt.float32
i16 = mybir.dt.int16

TRACE = False
LAST = {}
add_op = mybir.AluOpType.add
mult_op = mybir.AluOpType.mult


def _pow2ceil(x):
    x = np.maximum(x, 1)
    return (2 ** np.ceil(np.log2(x))).astype(np.int64)


def _wrap16(arr):
    """[S] int -> [128, S/16] int16 dma_gather index layout (index i at
    partition i%16, col i//16; replicated to all 8 Q7 cores)."""
    n = arr.shape[0]
    assert n % 16 == 0
    a = arr.reshape(n // 16, 16).T.astype(np.int16)
    return np.ascontiguousarray(np.tile(a, (8, 1)))


def _build_schedule(src, dst):
    deg = np.bincount(dst, minlength=N).astype(np.int64)
    in_a = src < 4 * SH
    d_a = np.bincount(dst[in_a], minlength=N).astype(np.int64)
    d_b = deg - d_a
    da_cap = _pow2ceil(d_a)
    db_cap = _pow2ceil(d_b)
    nodecore = np.arange(N) // SH

    n_ad = {}
    n_bd = {}
    for d in DS:
        g = P // d
        ca = max(int(((da_cap == d) & (nodecore == c)).sum()) for c in range(NC))
        cb = max(int(((db_cap == d) & (d_b > 0) & (nodecore == c)).sum())
                 for c in range(NC))
        n_ad[d] = math.ceil(ca / g) * g if ca else 0
        n_bd[d] = math.ceil(cb / g) * g if cb else 0

    def build_layout(n_d):
        # tile col0s avoiding 512-boundary crossings; per-class column
        # position lists (tile-major order)
        tiles_d = []
        colpos = {}
        cur = 0
        for d in DS:
            nd = n_d[d]
            if nd == 0:
                continue
            g = P // d
            pos = []
            for t in range(nd // g):
                if cur // 512 != (cur + g - 1) // 512:
                    cur = (cur // 512 + 1) * 512
                tiles_d.append((d, cur))
                pos.extend(range(cur, cur + g))
                cur += g
            colpos[d] = pos
        return tiles_d, colpos, cur

    tilesA, colposA, C = build_layout(n_ad)
    tilesB, colposB, CB = build_layout(n_bd)
    C_pad = math.ceil(C / P) * P
    CB_pad = max(P, math.ceil(CB / P) * P)
    assert C_pad <= CHUNK_ROWS - P, C_pad
    ZROW = C_pad                                 # statically-zeroed row

    tiles = ([("A", d, c0) for d, c0 in tilesA] +
             [("B", d, c0) for d, c0 in tilesB])
    t_a = len(tilesA)
    t_b = len(tilesB)

    order = np.argsort(dst, kind="stable")
    src_sorted = src[order]
    bounds = np.searchsorted(dst, np.arange(N + 1), sorter=order)

    # ---- global column assignment (A-order per core) ----
    col_of = np.full(N, -1, np.int64)
    colsA_all = []
    for c in range(NC):
        nodes = np.arange(c * SH, (c + 1) * SH)
        cols = np.full(C, -1, np.int64)
        for d in DS:
            if n_ad[d] == 0:
                continue
            sel = nodes[da_cap[nodes] == d]
            pos = np.asarray(colposA[d][:len(sel)], np.int64)
            cols[pos] = sel
        valid = cols >= 0
        col_of[cols[valid]] = np.nonzero(valid)[0]
        colsA_all.append(cols)
    row_of = nodecore * CHUNK_ROWS + col_of

    per_core = []
    for c in range(NC):
        colsA = colsA_all[c]

        slotsA = np.full(t_a * P, ZROW, np.int64)
        slot = 0
        for d in DS:
            nd = n_ad[d]
            if nd == 0:
                continue
            g = P // d
            cpos = colposA[d]
            for i in range(nd):
                n = colsA[cpos[i]]
                if n >= 0:
                    e0, e1 = bounds[n], bounds[n + 1]
                    ss = src_sorted[e0:e1]
                    ss = ss[ss < 4 * SH]
                    assert len(ss) <= d
                    slotsA[slot:slot + len(ss)] = row_of[ss]
                slot += d
                if (i + 1) % g == 0:
                    slot += P - g * d
        assert slot == t_a * P

        colsB = np.full(CB, -1, np.int64)
        posB = {}
        for d in DS:
            if n_bd[d] == 0:
                continue
            sel = np.arange(c * SH, (c + 1) * SH)
            sel = sel[(db_cap[sel] == d) & (d_b[sel] > 0)]
            cpos = colposB[d]
            for j, n in enumerate(sel):
                colsB[cpos[j]] = n
                posB[n] = cpos[j]
        slotsB = np.full(t_b * P, ZROW, np.int64)
        slot = 0
        for d in DS:
            nd = n_bd[d]
            if nd == 0:
                continue
            g = P // d
            cpos = colposB[d]
            for i in range(nd):
                n = colsB[cpos[i]]
                if n >= 0:
                    e0, e1 = bounds[n], bounds[n + 1]
                    ss = src_sorted[e0:e1]
                    ss = ss[ss >= 4 * SH]
                    assert 0 < len(ss) <= d
                    slotsB[slot:slot + len(ss)] = row_of[ss] - 4 * CHUNK_ROWS
                slot += d
                if (i + 1) % g == 0:
                    slot += P - g * d
        assert slot == t_b * P

        fix = np.full(C_pad, CB, np.int64)       # default -> zero scratch row
        for col in range(C):
            n = colsA[col]
            if n >= 0 and n in posB:
                fix[col] = posB[n]

        per_core.append(dict(slotsA=slotsA, slotsB=slotsB, fix=fix,
                             colsA=colsA))

    meta = dict(n_ad=n_ad, n_bd=n_bd, C=C, C_pad=C_pad, CB=CB, CB_pad=CB_pad,
                ZROW=ZROW, tiles=tiles, t_a=t_a, t_b=t_b, deg=deg)
    return meta, per_core


def _host_inputs(meta, per_core, inputs):
    x = np.asarray(inputs["x"], np.float32)
    batch = np.asarray(inputs["batch"])
    C_pad = meta["C_pad"]
    counts = np.bincount(batch, minlength=G).astype(np.float32)
    cdiv = 1.0 / np.maximum(counts, 1.0)
    deg = meta["deg"].astype(np.float32)
    dinv_n = np.where(deg > 0, 1.0 / np.sqrt(deg), 0.0).astype(np.float32)

    def catk(w):                                  # [K, fin, H] -> [fin, K*H]
        return np.ascontiguousarray(np.concatenate(list(w), axis=1))

    def blockdiag(w):                             # [K, H, H] -> [KH, KH]
        o = np.zeros((FEAT, FEAT), np.float32)
        for k in range(KS):
            o[k * H:(k + 1) * H, k * H:(k + 1) * H] = w[k]
        return o

    shared = {}
    for li in range(3):
        s = 0.5 if li > 0 else 1.0
        shared[f"wi{li}"] = catk(np.asarray(inputs[f"init_w{li+1}"], np.float32)) * s
        shared[f"wr{li}"] = catk(np.asarray(inputs[f"root_w{li+1}"], np.float32)) * s
        shared[f"wbd{li}"] = blockdiag(np.asarray(inputs[f"w{li+1}"], np.float32))
        shared[f"bb{li}"] = np.ascontiguousarray(
            np.asarray(inputs[f"b{li+1}"], np.float32).reshape(KS * H, 1))
    shared["linw"] = np.ascontiguousarray(np.asarray(inputs["lin_w"], np.float32))
    shared["linb"] = np.ascontiguousarray(
        np.tile(np.asarray(inputs["lin_b"], np.float32).reshape(1, OUT), (G, 1)))
    shared["ident"] = np.eye(P, dtype=np.float32)
    shared["fold"] = np.ascontiguousarray(
        np.vstack([np.eye(H, dtype=np.float32), np.eye(H, dtype=np.float32)]))
    for d in DS:
        if meta["n_ad"][d] == 0 and meta["n_bd"][d] == 0:
            continue
        g = P // d
        pat = np.zeros((P, g), np.float32)
        for j in range(g):
            pat[j * d:(j + 1) * d, j] = 1.0
        shared[f"pat{d}"] = pat.astype(ml_dtypes.bfloat16)

    in_maps = []
    for c in range(NC):
        pc = per_core[c]
        cols = pc["colsA"]
        xT = np.zeros((F_IN, C_pad), np.float32)
        dv = np.zeros((1, C_pad), np.float32)
        pp = np.zeros((C_pad, G), np.float32)
        valid = cols >= 0
        vc = np.nonzero(valid)[0]
        vn = cols[valid]
        xT[:, vc] = x[vn].T
        dv[0, vc] = dinv_n[vn]
        pp[vc, batch[vn]] = 0.5 * cdiv[batch[vn]]
        m = dict(shared)
        m["xT"] = xT
        m["dinv"] = np.ascontiguousarray(np.tile(dv, (P, 1)))
        m["poolP"] = pp
        m["idxA"] = _wrap16(pc["slotsA"])
        m["idxB"] = _wrap16(pc["slotsB"])
        m["idxF"] = _wrap16(pc["fix"])
        in_maps.append(m)
    return in_maps


# ---------------------- numpy mirror of the device program ------------------

def _numpy_forward(meta, in_maps):
    C_pad, CB, CB_pad = meta["C_pad"], meta["CB"], meta["CB_pad"]
    t_a, t_b, tiles = meta["t_a"], meta["t_b"], meta["tiles"]

    def to_bf(a):
        return np.asarray(a.astype(ml_dtypes.bfloat16), np.float32)

    xs = []
    for m in in_maps:
        xb = np.zeros((FEAT, C_pad), np.float32)
        xb[:F_IN] = m["xT"]
        xs.append(xb)
    table = np.zeros((NC * CHUNK_ROWS, FEAT), np.float32)

    def allgather(tabs):
        for c in range(NC):
            tb = np.zeros((CHUNK_ROWS, FEAT), np.float32)
            tb[:C_pad] = to_bf(tabs[c]).T
            table[c * CHUNK_ROWS:(c + 1) * CHUNK_ROWS] = tb

    def gather_reduce(c):
        m = in_maps[c]
        aggA = np.zeros((FEAT, C_pad), np.float32)
        aggB = np.zeros((FEAT, CB_pad), np.float32)
        for seq, idxw, agg, lo in (("A", m["idxA"], aggA, 0),
                                   ("B", m["idxB"], aggB, 4 * CHUNK_ROWS)):
            idx = idxw[:16].T.reshape(-1)
            win = table[lo:lo + 4 * CHUNK_ROWS]
            gathered = win[idx]
            ti = 0
            for s, d, col0 in tiles:
                if s != seq:
                    continue
                gsz = P // d
                blk = gathered[ti * P:(ti + 1) * P]
                for j in range(gsz):
                    agg[:, col0 + j] += blk[j * d:(j + 1) * d].sum(axis=0)
                ti += 1
        scratch = to_bf(np.ascontiguousarray(aggB.T))    # [CB_pad, FEAT]
        fix = m["idxF"][:16].T.reshape(-1)
        aggA += scratch[fix].T
        return aggA

    for li in range(3):
        tabs = []
        rootbs = []
        for c in range(NC):
            m = in_maps[c]
            xin = xs[c][:F_IN]
            rootbs.append(m[f"wr{li}"].T @ xin + m[f"bb{li}"])
            tabs.append((m[f"wi{li}"].T @ xin) * m["dinv"])
        allgather(tabs)
        for t in range(TS):
            aggs = [gather_reduce(c) for c in range(NC)]
            if t == 0:
                tabs = []
                for c in range(NC):
                    m = in_maps[c]
                    o = np.maximum(aggs[c] * m["dinv"] + rootbs[c], 0.0)
                    xs[c] = o
                    tabs.append(o * m["dinv"])
                allgather(tabs)
            else:
                for c in range(NC):
                    m = in_maps[c]
                    z = aggs[c] * m["dinv"]
                    o = np.maximum(m[f"wbd{li}"].T @ z + rootbs[c], 0.0)
                    o[:H] += o[H:]
                    xs[c] = o
    pooled = np.zeros((H, G), np.float32)
    for c in range(NC):
        pooled += xs[c][:H] @ in_maps[c]["poolP"]
    return pooled.T @ in_maps[0]["linw"] + in_maps[0]["linb"]


# ------------------------------ device program ------------------------------

def _build_program(meta):
    n_ad, n_bd = meta["n_ad"], meta["n_bd"]
    C, C_pad = meta["C"], meta["C_pad"]
    CB, CB_pad = meta["CB"], meta["CB_pad"]
    tiles, t_a, t_b = meta["tiles"], meta["t_a"], meta["t_b"]
    NBLK = C_pad // P
    NBLKB = CB_pad // P
    relu = mybir.ActivationFunctionType.Relu

    nc = bacc.Bacc("TRN2", target_bir_lowering=False, debug=False,
                   num_devices=NC, num_swdge_queues=4)

    par = {}

    def dp(name, shape, dt):
        par[name] = nc.declare_dram_parameter(name, list(shape), dt,
                                              isOutput=False)

    dp("xT", (F_IN, C_pad), f32)
    dp("dinv", (P, C_pad), f32)
    dp("poolP", (C_pad, G), f32)
    dp("idxA", (P, t_a * 8), i16)
    dp("idxB", (P, t_b * 8), i16)
    dp("idxF", (P, C_pad // 16), i16)
    dp("ident", (P, P), f32)
    dp("fold", (FEAT, H), f32)
    for li in range(3):
        dp(f"wi{li}", (F_IN, FEAT), f32)
        dp(f"wr{li}", (F_IN, FEAT), f32)
        dp(f"wbd{li}", (FEAT, FEAT), f32)
        dp(f"bb{li}", (FEAT, 1), f32)
    dp("linw", (H, OUT), f32)
    dp("linb", (G, OUT), f32)
    used_ds = [d for d in DS if n_ad[d] or n_bd[d]]
    for d in used_ds:
        dp(f"pat{d}", (P, P // d), bf16)
    out_ext = nc.declare_dram_parameter("out", [G, OUT], f32, isOutput=True)

    with tile.TileContext(nc) as tc:
        import contextlib
        stack = contextlib.ExitStack()
        dram = stack.enter_context(tc.tile_pool(name="dram", bufs=1, space="DRAM"))
        const = stack.enter_context(tc.tile_pool(name="const", bufs=1))
        sb = stack.enter_context(tc.tile_pool(name="sbufmain", bufs=1))
        stage_p = stack.enter_context(tc.tile_pool(name="stage", bufs=2))
        gq_p = stack.enter_context(tc.tile_pool(name="gq", bufs=4))
        ps_agg = stack.enter_context(tc.tile_pool(name="psagg", bufs=2, space="PSUM"))
        ps_dense = stack.enter_context(tc.tile_pool(name="psdense", bufs=2, space="PSUM"))
        ps_tr = stack.enter_context(tc.tile_pool(name="pstr", bufs=2, space="PSUM"))
        ps_one = stack.enter_context(tc.tile_pool(name="psone", bufs=2, space="PSUM"))

        contrib = dram.tile([CHUNK_ROWS, FEAT], bf16, name="contrib")
        KREP = int(os.environ.get("KREP", "1"))
        tables = [dram.tile([NC * CHUNK_ROWS, FEAT], bf16, addr_space="Shared",
                            name=f"table{i}") for i in range(6 * KREP)]
        scratchB = dram.tile([CB_pad, FEAT], bf16, name="scratchB")
        ar_in = dram.tile([H, G], f32, name="ar_in")
        ar_out = dram.tile([H, G], f32, addr_space="Shared", name="ar_out")

        # ---- constants ----
        pats = {}
        for d in used_ds:
            t = const.tile([P, P // d], bf16, name=f"pat{d}_sb")
            nc.sync.dma_start(out=t[:], in_=par[f"pat{d}"][:])
            pats[d] = t
        w_sb = {}
        for li in range(3):
            for nm, shp in ((f"wi{li}", (F_IN, FEAT)), (f"wr{li}", (F_IN, FEAT)),
                            (f"wbd{li}", (FEAT, FEAT)), (f"bb{li}", (FEAT, 1))):
                t = const.tile(list(shp), f32, name=nm + "_sb")
                nc.sync.dma_start(out=t[:], in_=par[nm][:])
                w_sb[nm] = t
        linw_sb = const.tile([H, OUT], f32, name="linw_sb")
        nc.sync.dma_start(out=linw_sb[:], in_=par["linw"][:])
        linb_sb = const.tile([G, OUT], f32, name="linb_sb")
        nc.sync.dma_start(out=linb_sb[:], in_=par["linb"][:])
        dinv_sb = const.tile([P, C_pad], f32, name="dinv_sb")
        nc.sync.dma_start(out=dinv_sb[:], in_=par["dinv"][:])
        identf = const.tile([P, P], f32, name="identf")
        nc.sync.dma_start(out=identf[:], in_=par["ident"][:])
        fold_sb = const.tile([FEAT, H], f32, name="fold_sb")
        nc.sync.dma_start(out=fold_sb[:], in_=par["fold"][:])
        ident = const.tile([P, P], bf16, name="identb")
        nc.vector.tensor_copy(ident[:], identf[:])

        xbuf = sb.tile([FEAT, C_pad], f32, name="xbuf")
        rootb = sb.tile([FEAT, C_pad], f32, name="rootb")
        aggA = sb.tile([FEAT, C_pad], f32, name="aggA")
        aggB = sb.tile([FEAT, CB_pad], f32, name="aggB")
        tab = sb.tile([FEAT, C_pad], bf16, name="tab")

        nc.vector.memset(xbuf[:], 0.0)
        nc.vector.memset(aggA[:], 0.0)
        nc.vector.memset(aggB[:], 0.0)

        zt = const.tile([P, FEAT], bf16, name="ztile")
        nc.vector.memset(zt[:], 0.0)
        r = C_pad
        while r < CHUNK_ROWS:
            nr = min(P, CHUNK_ROWS - r)
            nc.sync.dma_start(out=contrib[r:r + nr, :], in_=zt[:nr, :])
            r += nr

        nc.sync.dma_start(out=xbuf[0:F_IN, :], in_=par["xT"][:])

        def dinv_bc(c0, c1):
            return dinv_sb[:, c0:c1]

        def dense_mm(wname, src_fn, post, addin=None):
            wt = w_sb[wname]
            for c0 in range(0, C_pad, 512):
                c1 = min(c0 + 512, C_pad)
                ps = ps_dense.tile([P, 512], f32, name="dense_ps",
                                   tag="dense_ps")
                if addin is None:
                    nc.tensor.matmul(out=ps[:, :c1 - c0], lhsT=wt[:],
                                     rhs=src_fn(c0, c1), start=True, stop=True)
                else:
                    # fold the elementwise addend in on the PE via an
                    # identity-matmul accumulate (keeps DVE off the path)
                    nc.tensor.matmul(out=ps[:, :c1 - c0], lhsT=wt[:],
                                     rhs=src_fn(c0, c1), start=True, stop=False)
                    nc.tensor.matmul(out=ps[:, :c1 - c0], lhsT=identf[:],
                                     rhs=addin(c0, c1), start=False, stop=True)
                post(ps, c0, c1)

        def transpose_to_rows(src_sb, nblk, dst_dram, idmat, psdt, dt,
                              stage_name):
            """dst_dram[b*128+p, :] = src_sb[:, b*128+p] for b < nblk.

            4 transposes share one PSUM tile so one wide DVE copy moves
            them; stage writes go out on the scalar HWDGE ring to keep the
            sync ring free for gather idx loads."""
            for b0 in range(0, nblk, 8):
                b1 = min(b0 + 8, nblk)
                st = stage_p.tile([P, 8 * P], dt, name=stage_name,
                                  tag=stage_name)
                for q0 in range(b0, b1, 4):
                    q1 = min(q0 + 4, b1)
                    pst = ps_tr.tile([P, 4 * P], psdt, name="tr_ps",
                                     tag="tr_ps")
                    for b in range(q0, q1):
                        nc.tensor.transpose(
                            out=pst[:, (b - q0) * P:(b - q0 + 1) * P],
                            in_=src_sb[:, b * P:(b + 1) * P],
                            identity=idmat[:])
                    nc.vector.tensor_copy(
                        st[:, (q0 - b0) * P:(q0 - b0 + q1 - q0) * P],
                        pst[:, :(q1 - q0) * P])
                dst = dst_dram[:].rearrange("(n p) e -> p n e", p=P)[:, b0:b1, :]
                nc.scalar.dma_start(
                    out=dst,
                    in_=st[:].rearrange("p (n e) -> p n e", e=P)[:, :b1 - b0, :])

        def write_table_and_ag(tbl):
            transpose_to_rows(tab, NBLK, contrib, ident, bf16, bf16, "tstage")
            nc.gpsimd.collective_compute(
                "AllGather", mybir.AluOpType.bypass,
                replica_groups=[list(range(NC))],
                ins=[contrib[:].opt()], outs=[tbl[:].opt()])

        qctr = [0]

        def run_seq(tbl, seq, tcount, idxp, lo, agg, cmax, accumulate):
            win_ap = tbl[lo:lo + 4 * CHUNK_ROWS, :]
            stiles = [x for x in tiles if x[0] == seq]

            def finalize(cur_ps, last_col0):
                pb0 = (last_col0 // 512) * 512
                pb1 = min(pb0 + 512, cmax)
                if accumulate:
                    nc.vector.tensor_tensor(agg[:, pb0:pb1], agg[:, pb0:pb1],
                                            cur_ps[:, :pb1 - pb0], add_op)
                else:
                    nc.vector.tensor_copy(agg[:, pb0:pb1], cur_ps[:, :pb1 - pb0])

            cur_ps = None
            for ch0 in range(0, tcount, GCH):
                ch1 = min(ch0 + GCH, tcount)
                n_idx = (ch1 - ch0) * P
                idxt = gq_p.tile([P, GCH * 8], i16, name=f"idx{seq}",
                                 tag="idxt")
                nc.sync.dma_start(
                    out=idxt[:, :n_idx // 16],
                    in_=idxp[:, ch0 * 8:ch0 * 8 + n_idx // 16])
                gst = gq_p.tile([P, GCH * P], bf16, name=f"gst{seq}",
                                tag="gst")
                nc.gpsimd.dma_gather(
                    gst[:, :n_idx].rearrange("p (b e) -> p b e", e=FEAT),
                    win_ap, idxt[:, :n_idx // 16], n_idx, n_idx, FEAT,
                    single_packet=False, queue_num=qctr[0] % 3)
                qctr[0] += 1
                if os.environ.get("KNOMM"):
                    continue
                for t in range(ch0, ch1):
                    _, d, col0 = stiles[t]
                    gsz = P // d
                    if cur_ps is None or col0 // 512 != stiles[t - 1][2] // 512:
                        if cur_ps is not None:
                            finalize(cur_ps, stiles[t - 1][2])
                        cur_ps = ps_agg.tile([P, 512], f32, name="agg_ps",
                                             tag="agg_ps")
                    nc.tensor.matmul(
                        out=cur_ps[:, col0 % 512:col0 % 512 + gsz],
                        lhsT=gst[:, (t - ch0) * P:(t - ch0 + 1) * P],
                        rhs=pats[d][:], start=True, stop=True)
            if cur_ps is not None:
                finalize(cur_ps, stiles[tcount - 1][2])

        def gather_reduce(tbl, PHASE=9):
            if PHASE < 3:
                run_seq(tbl, "A", t_a, par["idxA"], 0, aggA, C,
                        accumulate=False)
                return
            # B window first: aggB, then permute it into aggA (copy) while
            # the A-window gathers stream; A psum windows then accumulate.
            run_seq(tbl, "B", t_b, par["idxB"], 4 * CHUNK_ROWS, aggB, CB,
                    accumulate=False)
            if PHASE >= 4:
                transpose_to_rows(aggB, NBLKB, scratchB, identf, f32, bf16,
                                  "bstage")
                for b0 in range(0, NBLK, FIXCH):
                    b1 = min(b0 + FIXCH, NBLK)
                    n_idx = (b1 - b0) * P
                    fxt = stage_p.tile([P, FIXCH * 8], i16, name="fixidx",
                                       tag="fixidx")
                    nc.scalar.dma_start(
                        out=fxt[:, :n_idx // 16],
                        in_=par["idxF"][:, b0 * 8:b0 * 8 + n_idx // 16])
                    fst = stage_p.tile([P, FIXCH * P], bf16, name="fixstage",
                                       tag="fixstage")
                    nc.gpsimd.dma_gather(
                        fst[:, :n_idx].rearrange("p (b e) -> p b e", e=FEAT),
                        scratchB[:], fxt[:, :n_idx // 16], n_idx, n_idx, FEAT,
                        single_packet=False, queue_num=3)
                    for q0 in range(b0, b1, 4):
                        q1 = min(q0 + 4, b1)
                        pst = ps_tr.tile([P, 4 * P], bf16, name="tr_ps",
                                         tag="tr_ps")
                        for b in range(q0, q1):
                            nc.tensor.transpose(
                                out=pst[:, (b - q0) * P:(b - q0 + 1) * P],
                                in_=fst[:, (b - b0) * P:(b - b0 + 1) * P],
                                identity=ident[:])
                        # Scalar engine does this copy: DVE is saturated by
                        # window finalizes around the fixup, Scalar is idle
                        nc.scalar.activation(
                            out=aggA[:, q0 * P:q1 * P],
                            in_=pst[:, :(q1 - q0) * P],
                            func=mybir.ActivationFunctionType.Identity)
            run_seq(tbl, "A", t_a, par["idxA"], 0, aggA, C,
                    accumulate=PHASE >= 4)

        # ---------------------------- layers ----------------------------
        PHASE = int(os.environ.get("KPHASE", "9"))
        agi = 0
        for rep in range(KREP):
          if rep > 0:
            nc.sync.dma_start(out=xbuf[0:F_IN, :], in_=par["xT"][:])
          for li in range(3):
              bb = w_sb[f"bb{li}"]

              def post_rootb(ps, c0, c1, bb=bb):
                  nc.vector.tensor_tensor(
                      rootb[:, c0:c1], ps[:, :c1 - c0],
                      bb[:, 0:1].to_broadcast([FEAT, c1 - c0]), add_op)

              def post_tab(ps, c0, c1):
                  nc.vector.tensor_tensor(tab[:, c0:c1], ps[:, :c1 - c0],
                                          dinv_bc(c0, c1), mult_op)

              xsrc = (lambda c0, c1: xbuf[0:F_IN, c0:c1])
              dense_mm(f"wi{li}", xsrc, post_tab)
              if PHASE >= 1:
                  write_table_and_ag(tables[agi])
              agi += 1
              dense_mm(f"wr{li}", xsrc, post_rootb)

              for t in range(TS):
                  if PHASE >= 2:
                      gather_reduce(tables[agi - 1], PHASE)
                  # per-512-block epilogue: each block only depends on its
                  # own A-window finalize, so this streams under the gather
                  # tail instead of barriering on the last window
                  for c0 in range(0, C_pad, 512):
                      c1 = min(c0 + 512, C_pad)
                      nc.vector.tensor_tensor(aggA[:, c0:c1], aggA[:, c0:c1],
                                              dinv_bc(c0, c1), mult_op)
                      if t == 0:
                          nc.vector.tensor_tensor(aggA[:, c0:c1],
                                                  aggA[:, c0:c1],
                                                  rootb[:, c0:c1], add_op)
                          nc.scalar.activation(out=xbuf[:, c0:c1],
                                               in_=aggA[:, c0:c1], func=relu)
                          nc.vector.tensor_tensor(tab[:, c0:c1],
                                                  xbuf[:, c0:c1],
                                                  dinv_bc(c0, c1), mult_op)
                  if t == 0:
                      if PHASE >= 1:
                          write_table_and_ag(tables[agi])
                      agi += 1
                  else:
                      def post_out2(ps, c0, c1):
                          nc.scalar.activation(out=xbuf[:, c0:c1],
                                               in_=ps[:, :c1 - c0], func=relu)

                      dense_mm(f"wbd{li}", (lambda c0, c1: aggA[:, c0:c1]),
                               post_out2,
                               addin=(lambda c0, c1: rootb[:, c0:c1]))
                      # fold K stacks: xbuf[0:H] = xbuf[0:H] + xbuf[H:]
                      for c0 in range(0, C_pad, 512):
                          c1 = min(c0 + 512, C_pad)
                          psf = ps_dense.tile([P, 512], f32, name="dense_ps",
                                              tag="dense_ps")
                          nc.tensor.matmul(out=psf[0:H, :c1 - c0],
                                           lhsT=fold_sb[:],
                                           rhs=xbuf[:, c0:c1],
                                           start=True, stop=True)
                          nc.vector.tensor_copy(xbuf[0:H, c0:c1],
                                                psf[0:H, :c1 - c0])

        # ------------------------- pool + head -------------------------
        pooled_ps = ps_one.tile([H, G], f32, name="pool_ps", tag="pool_ps")
        for b in range(NBLK):
            pst = ps_tr.tile([P, P], f32, name="tr_ps", tag="tr_ps")
            nc.tensor.transpose(out=pst[:, 0:H],
                                in_=xbuf[0:H, b * P:(b + 1) * P],
                                identity=identf[0:H, 0:H])
            h3n = stage_p.tile([P, H], f32, name="h3n", tag="h3n")
            nc.vector.tensor_copy(h3n[:], pst[:, 0:H])
            ppt = stage_p.tile([P, G], f32, name="ppt", tag="ppt")
            nc.sync.dma_start(out=ppt[:], in_=par["poolP"][b * P:(b + 1) * P, :])
            nc.tensor.matmul(out=pooled_ps[:], lhsT=h3n[:], rhs=ppt[:],
                             start=(b == 0), stop=(b == NBLK - 1))
        pooled_sb = sb.tile([H, G], f32, name="pooled_sb")
        nc.vector.tensor_copy(pooled_sb[:], pooled_ps[:])
        nc.sync.dma_start(out=ar_in[:], in_=pooled_sb[:])
        nc.gpsimd.collective_compute(
            "AllReduce", mybir.AluOpType.add,
            replica_groups=[list(range(NC))],
            ins=[ar_in[:].opt()], outs=[ar_out[:].opt()])
        nc.sync.dma_start(out=pooled_sb[:], in_=ar_out[:])
        final_ps = ps_one.tile([G, OUT], f32, name="final_ps", tag="pool_ps")
        nc.tensor.matmul(out=final_ps[:], lhsT=pooled_sb[:], rhs=linw_sb[:],
                         start=True, stop=True)
        res_sb = sb.tile([G, OUT], f32, name="res_sb")
        nc.vector.tensor_tensor(res_sb[:], final_ps[:],
                                linb_sb[:], add_op)
        nc.sync.dma_start(out=out_ext[:], in_=res_sb[:])
        stack.close()

    nc.compile()
    return nc


def kernel(**inputs):
    src = np.asarray(inputs["edge_index"])[0].astype(np.int64)
    dst = np.asarray(inputs["edge_index"])[1].astype(np.int64)
    meta, per_core = _build_schedule(src, dst)
    in_maps = _host_inputs(meta, per_core, inputs)
    nc = _build_program(meta)
    res = run_bass_kernel_spmd(nc, in_maps, core_ids=list(range(NC)),
                               trace=TRACE)
    LAST["exec_time_ns"] = res.exec_time_ns
    LAST["res"] = res
    return np.asarray(res.results[0]["out"], np.float32)



# revision 37
# speedup vs baseline: 1.0921x; 1.0921x over previous
"""ARMA GNN (3x ARMAConv K=2,T=2 + global mean pool + linear) on 8 trn2
NeuronCores.

Strategy (dst-sharded message passing with a replicated bf16 feature table):
  - Nodes sharded by dst across 8 cores (6250 each). Each inner ARMA
    iteration rebuilds a [65536, 128] bf16 node-feature table (rows
    pre-scaled by dinv[src]) via AllGather of per-core [8192, 128] chunks.
  - Per-core aggregation: dma_gather of the in-edge source rows (slot
    schedule built on host: per dst node, its edges padded to a pow2 run
    degree D; nodes grouped into equal-D runs so a static [128, 128/D]
    block-ones bf16 pattern reduces each 128-slot tile with one
    TensorEngine matmul into per-node PSUM columns).
  - gcn norm factorized: dinv[src] pre-scale (table), dinv[dst] post-scale.
  - dma_gather indices are int16, so sources are split into two 32768-row
    windows (cores 0-3 / 4-7); each window has its own run schedule. The
    window-B partial aggregate (in B-run column order) is transposed to an
    HBM scratch and gathered back in A-column order, then added.
  - Dense matmuls keep features on partitions (T-layout), weights as lhsT.
  - Mean pool via matmul with host-built (0.5/count)-weighted pool matrix,
    AllReduce, small linear head.
"""
import math
import os

import numpy as np
import ml_dtypes

import concourse.bacc as bacc
import concourse.mybir as mybir
import concourse.tile as tile
from concourse.bass_utils import run_bass_kernel_spmd

N = 50000
E = 800000
G = 64
F_IN = 64
H = 64
KS = 2
TS = 2
OUT = 24
NC = 8
SH = N // NC
P = 128
FEAT = KS * H          # 128
CHUNK_ROWS = 6912
DS = [1, 2, 4, 8, 16, 32, 64, 128]
GCH = 14               # gather chunk, in 128-slot tiles
FIXCH = 8              # fixup gather chunk, in 128-col blocks

bf16 = mybir.dt.bfloat16
f32 = mybir.dt.float32
i16 = mybir.dt.int16

TRACE = False
LAST = {}
add_op = mybir.AluOpType.add
mult_op = mybir.AluOpType.mult


def _pow2ceil(x):
    x = np.maximum(x, 1)
    return (2 ** np.ceil(np.log2(x))).astype(np.int64)


def _wrap16(arr):
    """[S] int -> [128, S/16] int16 dma_gather index layout (index i at
    partition i%16, col i//16; replicated to all 8 Q7 cores)."""
    n = arr.shape[0]
    assert n % 16 == 0
    a = arr.reshape(n // 16, 16).T.astype(np.int16)
    return np.ascontiguousarray(np.tile(a, (8, 1)))


def _build_schedule(src, dst):
    deg = np.bincount(dst, minlength=N).astype(np.int64)
    in_a = src < 4 * SH
    d_a = np.bincount(dst[in_a], minlength=N).astype(np.int64)
    d_b = deg - d_a
    da_cap = _pow2ceil(d_a)
    db_cap = _pow2ceil(d_b)
    nodecore = np.arange(N) // SH

    n_ad = {}
    n_bd = {}
    for d in DS:
        g = P // d
        ca = max(int(((da_cap == d) & (nodecore == c)).sum()) for c in range(NC))
        cb = max(int(((db_cap == d) & (d_b > 0) & (nodecore == c)).sum())
                 for c in range(NC))
        n_ad[d] = math.ceil(ca / g) * g if ca else 0
        n_bd[d] = math.ceil(cb / g) * g if cb else 0

    def build_layout(n_d):
        # tile col0s avoiding 512-boundary crossings; per-class column
        # position lists (tile-major order)
        tiles_d = []
        colpos = {}
        cur = 0
        for d in DS:
            nd = n_d[d]
            if nd == 0:
                continue
            g = P // d
            pos = []
            for t in range(nd // g):
                if cur // 512 != (cur + g - 1) // 512:
                    cur = (cur // 512 + 1) * 512
                tiles_d.append((d, cur))
                pos.extend(range(cur, cur + g))
                cur += g
            colpos[d] = pos
        return tiles_d, colpos, cur

    tilesA, colposA, C = build_layout(n_ad)
    tilesB, colposB, CB = build_layout(n_bd)
    C_pad = math.ceil(C / P) * P
    CB_pad = max(P, math.ceil(CB / P) * P)
    assert C_pad <= CHUNK_ROWS - P, C_pad
    ZROW = C_pad                                 # statically-zeroed row

    tiles = ([("A", d, c0) for d, c0 in tilesA] +
             [("B", d, c0) for d, c0 in tilesB])
    t_a = len(tilesA)
    t_b = len(tilesB)

    order = np.argsort(dst, kind="stable")
    src_sorted = src[order]
    bounds = np.searchsorted(dst, np.arange(N + 1), sorter=order)

    # ---- global column assignment (A-order per core) ----
    col_of = np.full(N, -1, np.int64)
    colsA_all = []
    for c in range(NC):
        nodes = np.arange(c * SH, (c + 1) * SH)
        cols = np.full(C, -1, np.int64)
        for d in DS:
            if n_ad[d] == 0:
                continue
            sel = nodes[da_cap[nodes] == d]
            pos = np.asarray(colposA[d][:len(sel)], np.int64)
            cols[pos] = sel
        valid = cols >= 0
        col_of[cols[valid]] = np.nonzero(valid)[0]
        colsA_all.append(cols)
    row_of = nodecore * CHUNK_ROWS + col_of

    per_core = []
    for c in range(NC):
        colsA = colsA_all[c]

        slotsA = np.full(t_a * P, ZROW, np.int64)
        slot = 0
        for d in DS:
            nd = n_ad[d]
            if nd == 0:
                continue
            g = P // d
            cpos = colposA[d]
            for i in range(nd):
                n = colsA[cpos[i]]
                if n >= 0:
                    e0, e1 = bounds[n], bounds[n + 1]
                    ss = src_sorted[e0:e1]
                    ss = ss[ss < 4 * SH]
                    assert len(ss) <= d
                    slotsA[slot:slot + len(ss)] = row_of[ss]
                slot += d
                if (i + 1) % g == 0:
                    slot += P - g * d
        assert slot == t_a * P

        colsB = np.full(CB, -1, np.int64)
        posB = {}
        for d in DS:
            if n_bd[d] == 0:
                continue
            sel = np.arange(c * SH, (c + 1) * SH)
            sel = sel[(db_cap[sel] == d) & (d_b[sel] > 0)]
            cpos = colposB[d]
            for j, n in enumerate(sel):
                colsB[cpos[j]] = n
                posB[n] = cpos[j]
        slotsB = np.full(t_b * P, ZROW, np.int64)
        slot = 0
        for d in DS:
            nd = n_bd[d]
            if nd == 0:
                continue
            g = P // d
            cpos = colposB[d]
            for i in range(nd):
                n = colsB[cpos[i]]
                if n >= 0:
                    e0, e1 = bounds[n], bounds[n + 1]
                    ss = src_sorted[e0:e1]
                    ss = ss[ss >= 4 * SH]
                    assert 0 < len(ss) <= d
                    slotsB[slot:slot + len(ss)] = row_of[ss] - 4 * CHUNK_ROWS
                slot += d
                if (i + 1) % g == 0:
                    slot += P - g * d
        assert slot == t_b * P

        fix = np.full(C_pad, CB, np.int64)       # default -> zero scratch row
        for col in range(C):
            n = colsA[col]
            if n >= 0 and n in posB:
                fix[col] = posB[n]

        per_core.append(dict(slotsA=slotsA, slotsB=slotsB, fix=fix,
                             colsA=colsA))

    meta = dict(n_ad=n_ad, n_bd=n_bd, C=C, C_pad=C_pad, CB=CB, CB_pad=CB_pad,
                ZROW=ZROW, tiles=tiles, t_a=t_a, t_b=t_b, deg=deg)
    return meta, per_core


def _host_inputs(meta, per_core, inputs):
    x = np.asarray(inputs["x"], np.float32)
    batch = np.asarray(inputs["batch"])
    C_pad = meta["C_pad"]
    counts = np.bincount(batch, minlength=G).astype(np.float32)
    cdiv = 1.0 / np.maximum(counts, 1.0)
    deg = meta["deg"].astype(np.float32)
    dinv_n = np.where(deg > 0, 1.0 / np.sqrt(deg), 0.0).astype(np.float32)

    def catk(w):                                  # [K, fin, H] -> [fin, K*H]
        return np.ascontiguousarray(np.concatenate(list(w), axis=1))

    def blockdiag(w):                             # [K, H, H] -> [KH, KH]
        o = np.zeros((FEAT, FEAT), np.float32)
        for k in range(KS):
            o[k * H:(k + 1) * H, k * H:(k + 1) * H] = w[k]
        return o

    shared = {}
    for li in range(3):
        s = 0.5 if li > 0 else 1.0
        shared[f"wi{li}"] = catk(np.asarray(inputs[f"init_w{li+1}"], np.float32)) * s
        shared[f"wr{li}"] = catk(np.asarray(inputs[f"root_w{li+1}"], np.float32)) * s
        shared[f"wbd{li}"] = blockdiag(np.asarray(inputs[f"w{li+1}"], np.float32))
        shared[f"bb{li}"] = np.ascontiguousarray(
            np.asarray(inputs[f"b{li+1}"], np.float32).reshape(KS * H, 1))
    shared["linw"] = np.ascontiguousarray(np.asarray(inputs["lin_w"], np.float32))
    shared["linb"] = np.ascontiguousarray(
        np.tile(np.asarray(inputs["lin_b"], np.float32).reshape(1, OUT), (G, 1)))
    shared["ident"] = np.eye(P, dtype=np.float32)
    shared["fold"] = np.ascontiguousarray(
        np.vstack([np.eye(H, dtype=np.float32), np.eye(H, dtype=np.float32)]))
    for d in DS:
        if meta["n_ad"][d] == 0 and meta["n_bd"][d] == 0:
            continue
        g = P // d
        pat = np.zeros((P, g), np.float32)
        for j in range(g):
            pat[j * d:(j + 1) * d, j] = 1.0
        shared[f"pat{d}"] = pat.astype(ml_dtypes.bfloat16)

    in_maps = []
    for c in range(NC):
        pc = per_core[c]
        cols = pc["colsA"]
        xT = np.zeros((F_IN, C_pad), np.float32)
        dv = np.zeros((1, C_pad), np.float32)
        pp = np.zeros((C_pad, G), np.float32)
        valid = cols >= 0
        vc = np.nonzero(valid)[0]
        vn = cols[valid]
        xT[:, vc] = x[vn].T
        dv[0, vc] = dinv_n[vn]
        pp[vc, batch[vn]] = 0.5 * cdiv[batch[vn]]
        m = dict(shared)
        m["xT"] = xT
        m["dinv"] = np.ascontiguousarray(np.tile(dv, (P, 1)))
        m["poolP"] = pp
        m["idxA"] = _wrap16(pc["slotsA"])
        m["idxB"] = _wrap16(pc["slotsB"])
        m["idxF"] = _wrap16(pc["fix"])
        in_maps.append(m)
    return in_maps


# ---------------------- numpy mirror of the device program ------------------

def _numpy_forward(meta, in_maps):
    C_pad, CB, CB_pad = meta["C_pad"], meta["CB"], meta["CB_pad"]
    t_a, t_b, tiles = meta["t_a"], meta["t_b"], meta["tiles"]

    def to_bf(a):
        return np.asarray(a.astype(ml_dtypes.bfloat16), np.float32)

    xs = []
    for m in in_maps:
        xb = np.zeros((FEAT, C_pad), np.float32)
        xb[:F_IN] = m["xT"]
        xs.append(xb)
    table = np.zeros((NC * CHUNK_ROWS, FEAT), np.float32)

    def allgather(tabs):
        for c in range(NC):
            tb = np.zeros((CHUNK_ROWS, FEAT), np.float32)
            tb[:C_pad] = to_bf(tabs[c]).T
            table[c * CHUNK_ROWS:(c + 1) * CHUNK_ROWS] = tb

    def gather_reduce(c):
        m = in_maps[c]
        aggA = np.zeros((FEAT, C_pad), np.float32)
        aggB = np.zeros((FEAT, CB_pad), np.float32)
        for seq, idxw, agg, lo in (("A", m["idxA"], aggA, 0),
                                   ("B", m["idxB"], aggB, 4 * CHUNK_ROWS)):
            idx = idxw[:16].T.reshape(-1)
            win = table[lo:lo + 4 * CHUNK_ROWS]
            gathered = win[idx]
            ti = 0
            for s, d, col0 in tiles:
                if s != seq:
                    continue
                gsz = P // d
                blk = gathered[ti * P:(ti + 1) * P]
                for j in range(gsz):
                    agg[:, col0 + j] += blk[j * d:(j + 1) * d].sum(axis=0)
                ti += 1
        scratch = to_bf(np.ascontiguousarray(aggB.T))    # [CB_pad, FEAT]
        fix = m["idxF"][:16].T.reshape(-1)
        aggA += scratch[fix].T
        return aggA

    for li in range(3):
        tabs = []
        rootbs = []
        for c in range(NC):
            m = in_maps[c]
            xin = xs[c][:F_IN]
            rootbs.append(m[f"wr{li}"].T @ xin + m[f"bb{li}"])
            tabs.append((m[f"wi{li}"].T @ xin) * m["dinv"])
        allgather(tabs)
        for t in range(TS):
            aggs = [gather_reduce(c) for c in range(NC)]
            if t == 0:
                tabs = []
                for c in range(NC):
                    m = in_maps[c]
                    o = np.maximum(aggs[c] * m["dinv"] + rootbs[c], 0.0)
                    xs[c] = o
                    tabs.append(o * m["dinv"])
                allgather(tabs)
            else:
                for c in range(NC):
                    m = in_maps[c]
                    z = aggs[c] * m["dinv"]
                    o = np.maximum(m[f"wbd{li}"].T @ z + rootbs[c], 0.0)
                    o[:H] += o[H:]
                    xs[c] = o
    pooled = np.zeros((H, G), np.float32)
    for c in range(NC):
        pooled += xs[c][:H] @ in_maps[c]["poolP"]
    return pooled.T @ in_maps[0]["linw"] + in_maps[0]["linb"]


# ------------------------------ device program ------------------------------

def _build_program(meta):
    n_ad, n_bd = meta["n_ad"], meta["n_bd"]
    C, C_pad = meta["C"], meta["C_pad"]
    CB, CB_pad = meta["CB"], meta["CB_pad"]
    tiles, t_a, t_b = meta["tiles"], meta["t_a"], meta["t_b"]
    NBLK = C_pad // P
    NBLKB = CB_pad // P
    relu = mybir.ActivationFunctionType.Relu

    nc = bacc.Bacc("TRN2", target_bir_lowering=False, debug=False,
                   num_devices=NC, num_swdge_queues=4)

    par = {}

    def dp(name, shape, dt):
        par[name] = nc.declare_dram_parameter(name, list(shape), dt,
                                              isOutput=False)

    dp("xT", (F_IN, C_pad), f32)
    dp("dinv", (P, C_pad), f32)
    dp("poolP", (C_pad, G), f32)
    dp("idxA", (P, t_a * 8), i16)
    dp("idxB", (P, t_b * 8), i16)
    dp("idxF", (P, C_pad // 16), i16)
    dp("ident", (P, P), f32)
    dp("fold", (FEAT, H), f32)
    for li in range(3):
        dp(f"wi{li}", (F_IN, FEAT), f32)
        dp(f"wr{li}", (F_IN, FEAT), f32)
        dp(f"wbd{li}", (FEAT, FEAT), f32)
        dp(f"bb{li}", (FEAT, 1), f32)
    dp("linw", (H, OUT), f32)
    dp("linb", (G, OUT), f32)
    used_ds = [d for d in DS if n_ad[d] or n_bd[d]]
    for d in used_ds:
        dp(f"pat{d}", (P, P // d), bf16)
    out_ext = nc.declare_dram_parameter("out", [G, OUT], f32, isOutput=True)

    with tile.TileContext(nc) as tc:
        import contextlib
        stack = contextlib.ExitStack()
        dram = stack.enter_context(tc.tile_pool(name="dram", bufs=1, space="DRAM"))
        const = stack.enter_context(tc.tile_pool(name="const", bufs=1))
        sb = stack.enter_context(tc.tile_pool(name="sbufmain", bufs=1))
        stage_p = stack.enter_context(tc.tile_pool(name="stage", bufs=2))
        gq_p = stack.enter_context(tc.tile_pool(name="gq", bufs=6))
        ps_agg = stack.enter_context(tc.tile_pool(name="psagg", bufs=3, space="PSUM"))
        ps_dense = stack.enter_context(tc.tile_pool(name="psdense", bufs=2, space="PSUM"))
        ps_tr = stack.enter_context(tc.tile_pool(name="pstr", bufs=2, space="PSUM"))
        ps_one = stack.enter_context(tc.tile_pool(name="psone", bufs=1, space="PSUM"))

        contrib = dram.tile([CHUNK_ROWS, FEAT], bf16, name="contrib")
        KREP = int(os.environ.get("KREP", "1"))
        tables = [dram.tile([NC * CHUNK_ROWS, FEAT], bf16, addr_space="Shared",
                            name=f"table{i}") for i in range(6 * KREP)]
        scratchB = dram.tile([CB_pad, FEAT], bf16, name="scratchB")
        ar_in = dram.tile([H, G], f32, name="ar_in")
        ar_out = dram.tile([H, G], f32, addr_space="Shared", name="ar_out")

        # ---- constants ----
        pats = {}
        for d in used_ds:
            t = const.tile([P, P // d], bf16, name=f"pat{d}_sb")
            nc.sync.dma_start(out=t[:], in_=par[f"pat{d}"][:])
            pats[d] = t
        w_sb = {}
        for li in range(3):
            for nm, shp in ((f"wi{li}", (F_IN, FEAT)), (f"wr{li}", (F_IN, FEAT)),
                            (f"wbd{li}", (FEAT, FEAT)), (f"bb{li}", (FEAT, 1))):
                t = const.tile(list(shp), f32, name=nm + "_sb")
                nc.sync.dma_start(out=t[:], in_=par[nm][:])
                w_sb[nm] = t
        linw_sb = const.tile([H, OUT], f32, name="linw_sb")
        nc.sync.dma_start(out=linw_sb[:], in_=par["linw"][:])
        linb_sb = const.tile([G, OUT], f32, name="linb_sb")
        nc.sync.dma_start(out=linb_sb[:], in_=par["linb"][:])
        dinv_sb = const.tile([P, C_pad], f32, name="dinv_sb")
        nc.sync.dma_start(out=dinv_sb[:], in_=par["dinv"][:])
        identf = const.tile([P, P], f32, name="identf")
        nc.sync.dma_start(out=identf[:], in_=par["ident"][:])
        fold_sb = const.tile([FEAT, H], f32, name="fold_sb")
        nc.sync.dma_start(out=fold_sb[:], in_=par["fold"][:])
        ident = const.tile([P, P], bf16, name="identb")
        nc.vector.tensor_copy(ident[:], identf[:])

        xbuf = sb.tile([FEAT, C_pad], f32, name="xbuf")
        rootb = sb.tile([FEAT, C_pad], f32, name="rootb")
        aggA = sb.tile([FEAT, C_pad], f32, name="aggA")
        aggB = sb.tile([FEAT, CB_pad], f32, name="aggB")
        tab = sb.tile([FEAT, C_pad], bf16, name="tab")

        nc.vector.memset(xbuf[:], 0.0)
        nc.vector.memset(aggA[:], 0.0)
        nc.vector.memset(aggB[:], 0.0)

        zt = const.tile([P, FEAT], bf16, name="ztile")
        nc.vector.memset(zt[:], 0.0)
        r = C_pad
        while r < CHUNK_ROWS:
            nr = min(P, CHUNK_ROWS - r)
            nc.sync.dma_start(out=contrib[r:r + nr, :], in_=zt[:nr, :])
            r += nr

        nc.sync.dma_start(out=xbuf[0:F_IN, :], in_=par["xT"][:])

        def dinv_bc(c0, c1):
            return dinv_sb[:, c0:c1]

        def dense_mm(wname, src_fn, post, addin=None):
            wt = w_sb[wname]
            for c0 in range(0, C_pad, 512):
                c1 = min(c0 + 512, C_pad)
                ps = ps_dense.tile([P, 512], f32, name="dense_ps",
                                   tag="dense_ps")
                if addin is None:
                    nc.tensor.matmul(out=ps[:, :c1 - c0], lhsT=wt[:],
                                     rhs=src_fn(c0, c1), start=True, stop=True)
                else:
                    # fold the elementwise addend in on the PE via an
                    # identity-matmul accumulate (keeps DVE off the path)
                    nc.tensor.matmul(out=ps[:, :c1 - c0], lhsT=wt[:],
                                     rhs=src_fn(c0, c1), start=True, stop=False)
                    nc.tensor.matmul(out=ps[:, :c1 - c0], lhsT=identf[:],
                                     rhs=addin(c0, c1), start=False, stop=True)
                post(ps, c0, c1)

        def transpose_to_rows(src_sb, nblk, dst_dram, idmat, psdt, dt,
                              stage_name):
            """dst_dram[b*128+p, :] = src_sb[:, b*128+p] for b < nblk.

            4 transposes share one PSUM tile so one wide DVE copy moves
            them; stage writes go out on the scalar HWDGE ring to keep the
            sync ring free for gather idx loads."""
            for b0 in range(0, nblk, 8):
                b1 = min(b0 + 8, nblk)
                st = stage_p.tile([P, 8 * P], dt, name=stage_name,
                                  tag=stage_name)
                for q0 in range(b0, b1, 4):
                    q1 = min(q0 + 4, b1)
                    pst = ps_tr.tile([P, 4 * P], psdt, name="tr_ps",
                                     tag="tr_ps")
                    for b in range(q0, q1):
                        nc.tensor.transpose(
                            out=pst[:, (b - q0) * P:(b - q0 + 1) * P],
                            in_=src_sb[:, b * P:(b + 1) * P],
                            identity=idmat[:])
                    nc.vector.tensor_copy(
                        st[:, (q0 - b0) * P:(q0 - b0 + q1 - q0) * P],
                        pst[:, :(q1 - q0) * P])
                dst = dst_dram[:].rearrange("(n p) e -> p n e", p=P)[:, b0:b1, :]
                nc.scalar.dma_start(
                    out=dst,
                    in_=st[:].rearrange("p (n e) -> p n e", e=P)[:, :b1 - b0, :])

        def write_table_and_ag(tbl):
            transpose_to_rows(tab, NBLK, contrib, ident, bf16, bf16, "tstage")
            nc.gpsimd.collective_compute(
                "AllGather", mybir.AluOpType.bypass,
                replica_groups=[list(range(NC))],
                ins=[contrib[:].opt()], outs=[tbl[:].opt()])

        qctr = [0]

        def run_seq(tbl, seq, tcount, idxp, lo, agg, cmax, accumulate):
            win_ap = tbl[lo:lo + 4 * CHUNK_ROWS, :]
            stiles = [x for x in tiles if x[0] == seq]

            def finalize(cur_ps, last_col0):
                pb0 = (last_col0 // 512) * 512
                pb1 = min(pb0 + 512, cmax)
                if accumulate:
                    nc.vector.tensor_tensor(agg[:, pb0:pb1], agg[:, pb0:pb1],
                                            cur_ps[:, :pb1 - pb0], add_op)
                else:
                    nc.vector.tensor_copy(agg[:, pb0:pb1], cur_ps[:, :pb1 - pb0])

            cur_ps = None
            for ch0 in range(0, tcount, GCH):
                ch1 = min(ch0 + GCH, tcount)
                n_idx = (ch1 - ch0) * P
                idxt = gq_p.tile([P, GCH * 8], i16, name=f"idx{seq}",
                                 tag="idxt")
                nc.sync.dma_start(
                    out=idxt[:, :n_idx // 16],
                    in_=idxp[:, ch0 * 8:ch0 * 8 + n_idx // 16])
                gst = gq_p.tile([P, GCH * P], bf16, name=f"gst{seq}",
                                tag="gst")
                nc.gpsimd.dma_gather(
                    gst[:, :n_idx].rearrange("p (b e) -> p b e", e=FEAT),
                    win_ap, idxt[:, :n_idx // 16], n_idx, n_idx, FEAT,
                    single_packet=False, queue_num=qctr[0] % 3)
                qctr[0] += 1
                if os.environ.get("KNOMM"):
                    continue
                for t in range(ch0, ch1):
                    _, d, col0 = stiles[t]
                    gsz = P // d
                    if cur_ps is None or col0 // 512 != stiles[t - 1][2] // 512:
                        if cur_ps is not None:
                            finalize(cur_ps, stiles[t - 1][2])
                        cur_ps = ps_agg.tile([P, 512], f32, name="agg_ps",
                                             tag="agg_ps")
                    nc.tensor.matmul(
                        out=cur_ps[:, col0 % 512:col0 % 512 + gsz],
                        lhsT=gst[:, (t - ch0) * P:(t - ch0 + 1) * P],
                        rhs=pats[d][:], start=True, stop=True)
            if cur_ps is not None:
                finalize(cur_ps, stiles[tcount - 1][2])

        def gather_reduce(tbl, PHASE=9):
            if PHASE < 3:
                run_seq(tbl, "A", t_a, par["idxA"], 0, aggA, C,
                        accumulate=False)
                return
            # B window first: aggB, then permute it into aggA (copy) while
            # the A-window gathers stream; A psum windows then accumulate.
            run_seq(tbl, "B", t_b, par["idxB"], 4 * CHUNK_ROWS, aggB, CB,
                    accumulate=False)
            if PHASE >= 4:
                transpose_to_rows(aggB, NBLKB, scratchB, identf, f32, bf16,
                                  "bstage")
                for b0 in range(0, NBLK, FIXCH):
                    b1 = min(b0 + FIXCH, NBLK)
                    n_idx = (b1 - b0) * P
                    fxt = stage_p.tile([P, FIXCH * 8], i16, name="fixidx",
                                       tag="fixidx")
                    nc.scalar.dma_start(
                        out=fxt[:, :n_idx // 16],
                        in_=par["idxF"][:, b0 * 8:b0 * 8 + n_idx // 16])
                    fst = stage_p.tile([P, FIXCH * P], bf16, name="fixstage",
                                       tag="fixstage")
                    nc.gpsimd.dma_gather(
                        fst[:, :n_idx].rearrange("p (b e) -> p b e", e=FEAT),
                        scratchB[:], fxt[:, :n_idx // 16], n_idx, n_idx, FEAT,
                        single_packet=False, queue_num=3)
                    for q0 in range(b0, b1, 4):
                        q1 = min(q0 + 4, b1)
                        pst = ps_tr.tile([P, 4 * P], bf16, name="tr_ps",
                                         tag="tr_ps")
                        for b in range(q0, q1):
                            nc.tensor.transpose(
                                out=pst[:, (b - q0) * P:(b - q0 + 1) * P],
                                in_=fst[:, (b - b0) * P:(b - b0 + 1) * P],
                                identity=ident[:])
                        nc.vector.tensor_copy(
                            aggA[:, q0 * P:q1 * P],
                            pst[:, :(q1 - q0) * P])
            run_seq(tbl, "A", t_a, par["idxA"], 0, aggA, C,
                    accumulate=PHASE >= 4)

        # ---------------------------- layers ----------------------------
        PHASE = int(os.environ.get("KPHASE", "9"))
        agi = 0
        for rep in range(KREP):
          if rep > 0:
            nc.sync.dma_start(out=xbuf[0:F_IN, :], in_=par["xT"][:])
          for li in range(3):
              bb = w_sb[f"bb{li}"]

              def post_rootb(ps, c0, c1, bb=bb):
                  nc.vector.tensor_tensor(
                      rootb[:, c0:c1], ps[:, :c1 - c0],
                      bb[:, 0:1].to_broadcast([FEAT, c1 - c0]), add_op)

              def post_tab(ps, c0, c1):
                  nc.vector.tensor_tensor(tab[:, c0:c1], ps[:, :c1 - c0],
                                          dinv_bc(c0, c1), mult_op)

              xsrc = (lambda c0, c1: xbuf[0:F_IN, c0:c1])
              dense_mm(f"wi{li}", xsrc, post_tab)
              if PHASE >= 1:
                  write_table_and_ag(tables[agi])
              agi += 1
              dense_mm(f"wr{li}", xsrc, post_rootb)

              for t in range(TS):
                  if PHASE >= 2:
                      gather_reduce(tables[agi - 1], PHASE)
                  # per-512-block epilogue: each block only depends on its
                  # own A-window finalize, so this streams under the gather
                  # tail instead of barriering on the last window
                  for c0 in range(0, C_pad, 512):
                      c1 = min(c0 + 512, C_pad)
                      nc.vector.tensor_tensor(aggA[:, c0:c1], aggA[:, c0:c1],
                                              dinv_bc(c0, c1), mult_op)
                      if t == 0:
                          nc.vector.tensor_tensor(aggA[:, c0:c1],
                                                  aggA[:, c0:c1],
                                                  rootb[:, c0:c1], add_op)
                          nc.scalar.activation(out=xbuf[:, c0:c1],
                                               in_=aggA[:, c0:c1], func=relu)
                          nc.vector.tensor_tensor(tab[:, c0:c1],
                                                  xbuf[:, c0:c1],
                                                  dinv_bc(c0, c1), mult_op)
                  if t == 0:
                      if PHASE >= 1:
                          write_table_and_ag(tables[agi])
                      agi += 1
                  else:
                      def post_out2(ps, c0, c1):
                          nc.scalar.activation(out=xbuf[:, c0:c1],
                                               in_=ps[:, :c1 - c0], func=relu)

                      dense_mm(f"wbd{li}", (lambda c0, c1: aggA[:, c0:c1]),
                               post_out2,
                               addin=(lambda c0, c1: rootb[:, c0:c1]))
                      # fold K stacks: xbuf[0:H] = xbuf[0:H] + xbuf[H:]
                      for c0 in range(0, C_pad, 512):
                          c1 = min(c0 + 512, C_pad)
                          psf = ps_dense.tile([P, 512], f32, name="dense_ps",
                                              tag="dense_ps")
                          nc.tensor.matmul(out=psf[0:H, :c1 - c0],
                                           lhsT=fold_sb[:],
                                           rhs=xbuf[:, c0:c1],
                                           start=True, stop=True)
                          nc.vector.tensor_copy(xbuf[0:H, c0:c1],
                                                psf[0:H, :c1 - c0])

        # ------------------------- pool + head -------------------------
        pooled_ps = ps_one.tile([H, G], f32, name="pool_ps", tag="pool_ps")
        for b in range(NBLK):
            pst = ps_tr.tile([P, P], f32, name="tr_ps", tag="tr_ps")
            nc.tensor.transpose(out=pst[:, 0:H],
                                in_=xbuf[0:H, b * P:(b + 1) * P],
                                identity=identf[0:H, 0:H])
            h3n = stage_p.tile([P, H], f32, name="h3n", tag="h3n")
            nc.vector.tensor_copy(h3n[:], pst[:, 0:H])
            ppt = stage_p.tile([P, G], f32, name="ppt", tag="ppt")
            nc.sync.dma_start(out=ppt[:], in_=par["poolP"][b * P:(b + 1) * P, :])
            nc.tensor.matmul(out=pooled_ps[:], lhsT=h3n[:], rhs=ppt[:],
                             start=(b == 0), stop=(b == NBLK - 1))
        pooled_sb = sb.tile([H, G], f32, name="pooled_sb")
        nc.vector.tensor_copy(pooled_sb[:], pooled_ps[:])
        nc.sync.dma_start(out=ar_in[:], in_=pooled_sb[:])
        nc.gpsimd.collective_compute(
            "AllReduce", mybir.AluOpType.add,
            replica_groups=[list(range(NC))],
            ins=[ar_in[:].opt()], outs=[ar_out[:].opt()])
        nc.sync.dma_start(out=pooled_sb[:], in_=ar_out[:])
        final_ps = ps_one.tile([G, OUT], f32, name="final_ps", tag="pool_ps")
        nc.tensor.matmul(out=final_ps[:], lhsT=pooled_sb[:], rhs=linw_sb[:],
                         start=True, stop=True)
        res_sb = sb.tile([G, OUT], f32, name="res_sb")
        nc.vector.tensor_tensor(res_sb[:], final_ps[:],
                                linb_sb[:], add_op)
        nc.sync.dma_start(out=out_ext[:], in_=res_sb[:])
        stack.close()

    nc.compile()
    return nc


def kernel(**inputs):
    src = np.asarray(inputs["edge_index"])[0].astype(np.int64)
    dst = np.asarray(inputs["edge_index"])[1].astype(np.int64)
    meta, per_core = _build_schedule(src, dst)
    in_maps = _host_inputs(meta, per_core, inputs)
    nc = _build_program(meta)
    res = run_bass_kernel_spmd(nc, in_maps, core_ids=list(range(NC)),
                               trace=TRACE)
    LAST["exec_time_ns"] = res.exec_time_ns
    LAST["res"] = res
    return np.asarray(res.results[0]["out"], np.float32)

